# revision 11
# baseline (speedup 1.0000x reference)
"""Trainium2 Bass kernel for nn_Encoder_Decoder_60576218743336.

Strategy (8 NeuronCores, SPMD):
- Transformer stacks: data-parallel over batch (4). Core c handles batch c%4
  (cores 4-7 duplicate batches 0-3 so the whole chip participates in the
  vocab-parallel generator afterwards).
- Activations are feature-major XT[feature, token] folded into SBUF tiles
  [128, 8*T]; every projection uses the stored weights [in, out] directly as
  the PE stationary operand (out = W.T @ XT). fp32r matmuls (TF32-like,
  full PE rate at free-dim >= 256).
- Attention: WQ/WK/WV columns permuted host-side to head-major so per-head
  slices are contiguous; scores computed in ST[k, q] layout so the softmax
  normalizer is a ones-column appended to V (no transposes; exp without
  max-subtraction -- exact vs the reference semantics, validated in numpy).
- LayerNorm over the feature (partition) axis via ones-matmul column sums,
  unbiased std, (std + eps) as in the reference.
- Generator: vocab-parallel over all 8 cores: AllGather y, each core computes
  logits for its 4000-wide vocab slice, distributed log-softmax via
  AllReduce(min of -max) + AllReduce(add) of per-token stats.

kernel(**inputs) takes the FULL inputs (named as reference.setup_inputs())
and returns the FULL (4, 256, 32000) float32 output.
"""

from contextlib import ExitStack

import numpy as np

import concourse.bacc as bacc
import concourse.bass as bass
import concourse.mybir as mybir
import concourse.tile as tile
from concourse.bass import ds, ts
from concourse.bass_utils import run_bass_kernel_spmd
from concourse.masks import make_identity

P = 128
L, ED, NH, QD, VD, FF, VS, VT = 4, 1024, 16, 64, 64, 4096, 32000, 32000
BS, SS, TS, PAD = 4, 512, 256, 0
EPS = 1e-6
SCALE = float(1.0 / np.sqrt(QD))
NEG = np.float32(-1e9 * SCALE)
EC = ED // P            # 8 feature chunks
FC = FF // P            # 32 ff chunks
SC = SS // P            # 4 src token chunks
TC = TS // P            # 2 tgt token chunks
NCORES = 8
VSL = VT // NCORES      # 4000 vocab columns per core
VCH = [512] * 7 + [VSL - 512 * 7]
NG = BS * TC            # 8 token-chunk groups in the generator

F32 = mybir.dt.float32
F32R = mybir.dt.float32r
I32 = mybir.dt.int32
AF = mybir.ActivationFunctionType
ALU = mybir.AluOpType
AX = mybir.AxisListType

HEAD_PERM = np.array([d * NH + h for h in range(NH) for d in range(QD)])
VROW = NH * (VD + 1)    # token-major V row: 16 heads x (64 vals + 1.0)

LAST_RESULTS = None


def build():
    nc = bacc.Bacc("TRN2", target_bir_lowering=False, debug=False,
                   num_devices=NCORES)

    def din(name, shape, dtype=F32R):
        return nc.dram_tensor(name, shape, dtype, kind="ExternalInput").ap()

    semb_g = din("semb_g", [SS, ED], F32)
    temb_g = din("temb_g", [TS, ED], F32)
    pe_cols = din("pe_cols", [EC, P], F32)
    encm_d = din("encm", [SC, P], F32)
    decm_d = din("decm8", [TC, P, TS], F32)
    gWs = din("gWs", [ED, VSL], F32R)
    gbs = din("gbs", [1, VSL], F32)

    W = {}
    for pfx in ("e", "d"):
        W[pfx + "Q"] = din(pfx + "Qp", [L, ED, ED])
        W[pfx + "K"] = din(pfx + "Kp", [L, ED, ED])
        W[pfx + "V"] = din(pfx + "Vp", [L, ED, ED])
        W[pfx + "O"] = din(pfx + "O", [L, ED, ED])
        W[pfx + "Fi"] = din(pfx + "Fi", [L, ED, FF])
        W[pfx + "Fo"] = din(pfx + "Fo", [L, FF, ED])
        W[pfx + "Qb"] = din(pfx + "Qbp", [L, EC, P], F32)
        W[pfx + "Kb"] = din(pfx + "Kbp", [L, EC, P], F32)
        W[pfx + "Vbr"] = din(pfx + "Vbr", [L, 1, ED], F32)
        W[pfx + "Ob"] = din(pfx + "Ob", [L, EC, P], F32)
        W[pfx + "Fib"] = din(pfx + "Fib", [L, FC, P], F32)
        W[pfx + "Fob"] = din(pfx + "Fob", [L, EC, P], F32)
        W[pfx + "g"] = din(pfx + "g", [L, 2 * EC, P], F32)

    out_d = nc.dram_tensor("out", [BS * TS, VSL], F32,
                           kind="ExternalOutput").ap()

    with ExitStack() as ctx:
        tc = ctx.enter_context(tile.TileContext(nc))

        const = ctx.enter_context(tc.tile_pool(name="const", bufs=1))
        psum = ctx.enter_context(tc.tile_pool(name="psum", bufs=8,
                                              space="PSUM"))
        rows = ctx.enter_context(tc.tile_pool(name="rows", bufs=1))
        arow = ctx.enter_context(tc.tile_pool(name="arow", bufs=2))
        cols = ctx.enter_context(tc.tile_pool(name="cols", bufs=2))
        dram = ctx.enter_context(tc.tile_pool(name="dram", bufs=1,
                                              space="DRAM"))

        onesf = const.tile([P, 1], F32, tag="onesf")
        nc.any.memset(onesf[:], 1.0)
        ones_col = const.tile([P, 1], F32R, tag="ones_col")
        nc.vector.tensor_copy(ones_col[:], onesf[:])
        onesrf = const.tile([1, P], F32, tag="onesrf")
        nc.any.memset(onesrf[:], 1.0)
        ones_row = const.tile([1, P], F32R, tag="ones_row")
        nc.vector.tensor_copy(ones_row[:], onesrf[:])
        ident = const.tile([P, P], F32, tag="ident")
        make_identity(nc, ident[:])
        eps_row = const.tile([1, 1], F32, tag="eps_row")
        nc.any.memset(eps_row[:], EPS)

        y_bounce = dram.tile([EC, P, TS], F32R, tag="yb")
        ag_out = dram.tile([NCORES * EC, P, TS], F32R, tag="ag")
        m_bounce = dram.tile([P, NG], F32, tag="mb")
        m_red = dram.tile([P, NG], F32, tag="mr")
        s_bounce = dram.tile([P, NG], F32, tag="sb")
        s_red = dram.tile([P, NG], F32, tag="sr")

        # ============================ helpers ===========================
        def proj(out_sb, x_sb, T, w_dram, bias_sb, bias_off, wk):
            """out[f, t] = W.T @ x + b  (feature-major, EC->EC chunks)."""
            accs = [psum.tile([P, 512], F32, tag="mm", name=f"pacc{m}") for m in range(EC)]
            for e in range(EC):
                wt = wk.tile([P, ED], F32R, tag="w")
                nc.sync.dma_start(wt[:], w_dram[ts(e, P), :])
                for m in range(EC):
                    nc.tensor.matmul(
                        accs[m][:, :T], wt[:, ts(m, P)],
                        x_sb[:, e * T:(e + 1) * T],
                        start=(e == 0), stop=(e == EC - 1))
            for m in range(EC):
                nc.vector.tensor_scalar_add(
                    out_sb[:, m * T:(m + 1) * T], accs[m][:, :T],
                    bias_sb[:, bias_off + m: bias_off + m + 1])

        def v_project(v_sb, x_sb, T, w_dram, vbB, wk):
            """token-major V with per-head ones column appended."""
            n_tc = T // P
            accs = [psum.tile([P, 512], F32, tag="mm", name=f"vacc{m}")
                    for m in range(n_tc * 2)]
            for e in range(EC):
                wt = wk.tile([P, ED], F32R, tag="w")
                nc.sync.dma_start(wt[:], w_dram[ts(e, P), :])
                for kc in range(n_tc):
                    for half in range(2):
                        nc.tensor.matmul(
                            accs[kc * 2 + half][:],
                            x_sb[:, e * T + kc * P: e * T + (kc + 1) * P],
                            wt[:, ts(half, 512)],
                            start=(e == 0), stop=(e == EC - 1))
            for kc in range(n_tc):
                for half in range(2):
                    base = kc * VROW + half * 8 * (VD + 1)
                    dst = v_sb[:, base: base + 8 * (VD + 1)].rearrange(
                        "p (h d) -> p h d", h=8)[:, :, :VD]
                    src3 = accs[kc * 2 + half][:].rearrange(
                        "p (h d) -> p h d", h=8)
                    vb3 = vbB[half][:].rearrange("p (h d) -> p h d", h=8)
                    nc.vector.tensor_tensor(dst, src3, vb3, op=ALU.add)
                ones_dst = v_sb[:, kc * VROW:(kc + 1) * VROW].rearrange(
                    "p (h d) -> p h d", h=NH)[:, :, VD:]
                nc.vector.tensor_copy(
                    ones_dst,
                    ones_col[:].unsqueeze(1).broadcast_to([P, NH, 1]))

        def attention(zt, qt, kt, v_sb, Tq, Tk, expp,
                      enc_mask=None, dec_mask=None):
            n_kc = Tk // P
            for h in range(NH):
                qp, qo = (h % 2) * 64, (h // 2) * Tq
                e_t = expp.tile([P, 4 * 512], F32R, tag="exp")
                for kc in range(n_kc):
                    st = psum.tile([P, 512], F32, tag="mm")
                    nc.tensor.matmul(
                        st[:, :Tq],
                        kt[qp:qp + 64,
                           (h // 2) * Tk + kc * P:(h // 2) * Tk + (kc + 1) * P],
                        qt[qp:qp + 64, qo:qo + Tq],
                        start=True, stop=True)
                    if dec_mask is not None:
                        nc.vector.tensor_tensor(
                            st[:, :Tq], st[:, :Tq],
                            dec_mask[:, kc * Tq:(kc + 1) * Tq], op=ALU.add)
                        nc.scalar.activation(
                            e_t[:, kc * Tq:(kc + 1) * Tq], st[:, :Tq],
                            AF.Exp, scale=SCALE)
                    else:
                        nc.scalar.activation(
                            e_t[:, kc * Tq:(kc + 1) * Tq], st[:, :Tq],
                            AF.Exp, bias=enc_mask[:, kc:kc + 1], scale=SCALE)
                zp = psum.tile([P, 512], F32, tag="mm")
                for kc in range(n_kc):
                    nc.tensor.matmul(
                        zp[:VD + 1, :Tq],
                        v_sb[:, kc * VROW + h * (VD + 1):
                             kc * VROW + (h + 1) * (VD + 1)],
                        e_t[:, kc * Tq:(kc + 1) * Tq],
                        start=(kc == 0), stop=(kc == n_kc - 1))
                rcp = arow.tile([1, 512], F32R, tag="rcp")
                with nc.allow_low_precision(reason="f32r recip, fp32-width"):
                    nc.vector.reciprocal(rcp[:, :Tq], zp[VD:VD + 1, :Tq])
                rb = psum.tile([P, 512], F32, tag="mm")
                nc.tensor.matmul(rb[:VD, :Tq], ones_row[:, :VD],
                                 rcp[:, :Tq], start=True, stop=True)
                rbs = arow.tile([VD, 512], F32, tag="rbs")
                nc.scalar.copy(rbs[:, :Tq], rb[:VD, :Tq])
                nc.vector.tensor_tensor(
                    zt[qp:qp + 64, qo:qo + Tq], zp[:VD, :Tq],
                    rbs[:, :Tq], op=ALU.mult)

        def ln(out_sb, a_sb, T, sq_sb, g_sb):
            """LN over 1024 features, per token; a/out folded [128, 8*T]."""
            nf = ED
            nc.vector.tensor_tensor(sq_sb[:, :EC * T], a_sb[:, :EC * T],
                                    a_sb[:, :EC * T], op=ALU.mult)
            psS = psum.tile([P, 512], F32, tag="mm")
            psQ = psum.tile([P, 512], F32, tag="mm")
            for c in range(EC):
                nc.tensor.matmul(psS[:1, :T], ones_col[:],
                                 a_sb[:, c * T:(c + 1) * T],
                                 start=(c == 0), stop=(c == EC - 1))
            for c in range(EC):
                nc.tensor.matmul(psQ[:1, :T], ones_col[:],
                                 sq_sb[:, c * T:(c + 1) * T],
                                 start=(c == 0), stop=(c == EC - 1))
            mu = rows.tile([1, 512], F32R, tag="mu")
            nc.scalar.mul(mu[:, :T], psS[:1, :T], 1.0 / nf)
            sA = rows.tile([1, 512], F32, tag="sA")
            nc.vector.tensor_tensor(sA[:, :T], mu[:, :T], mu[:, :T],
                                    op=ALU.mult)
            nc.scalar.mul(sA[:, :T], sA[:, :T], float(nf) / (nf - 1))
            sB = rows.tile([1, 512], F32, tag="sB")
            nc.scalar.mul(sB[:, :T], psQ[:1, :T], 1.0 / (nf - 1))
            nc.vector.tensor_tensor(sB[:, :T], sB[:, :T], sA[:, :T],
                                    op=ALU.subtract)
            nc.scalar.activation(sB[:, :T], sB[:, :T], AF.Sqrt)
            nc.scalar.activation(sB[:, :T], sB[:, :T], AF.Identity,
                                 bias=eps_row[:])
            rstd = rows.tile([1, 512], F32R, tag="rstd")
            with nc.allow_low_precision(reason="f32r recip, fp32-width"):
                nc.vector.reciprocal(rstd[:, :T], sB[:, :T])
            muB = psum.tile([P, 512], F32, tag="mm")
            rsB = psum.tile([P, 512], F32, tag="mm")
            nc.tensor.matmul(muB[:, :T], ones_row[:], mu[:, :T],
                             start=True, stop=True)
            nc.tensor.matmul(rsB[:, :T], ones_row[:], rstd[:, :T],
                             start=True, stop=True)
            a3 = a_sb[:, :EC * T].rearrange("p (c t) -> p c t", c=EC)
            o3 = out_sb[:, :EC * T].rearrange("p (c t) -> p c t", c=EC)
            muB3 = muB[:, :T].unsqueeze(1).broadcast_to([P, EC, T])
            rsB3 = rsB[:, :T].unsqueeze(1).broadcast_to([P, EC, T])
            nc.vector.tensor_tensor(o3, a3, muB3, op=ALU.subtract)
            nc.vector.tensor_tensor(o3, o3, rsB3, op=ALU.mult)
            for c in range(EC):
                nc.scalar.activation(
                    out_sb[:, c * T:(c + 1) * T], out_sb[:, c * T:(c + 1) * T],
                    AF.Identity,
                    bias=g_sb[:, EC + c: EC + c + 1],
                    scale=g_sb[:, c:c + 1])

        def ffn(out_acc, t_sb, T, fi_dram, fib_sb, fib_off, fo_dram, fob_sb,
                wfi, wfo, hpool):
            halves = [(0, T)] if T <= 256 else [(0, 256), (256, 256)]
            gsz = 8
            for g in range(FC // gsz):
                for h0, hT in halves:
                    hps = [psum.tile([P, 512], F32, tag="mm", name=f"hp{m}")
                           for m in range(gsz)]
                    for e in range(EC):
                        wt = wfi.tile([P, gsz * P], F32R, tag="wfi")
                        nc.sync.dma_start(
                            wt[:], fi_dram[ts(e, P), ds(g * gsz * P, gsz * P)])
                        for fg in range(gsz):
                            nc.tensor.matmul(
                                hps[fg][:, :hT], wt[:, ts(fg, P)],
                                t_sb[:, e * T + h0: e * T + h0 + hT],
                                start=(e == 0), stop=(e == EC - 1))
                    hts = []
                    for fg in range(gsz):
                        f = g * gsz + fg
                        ht = hpool.tile([P, 256], F32R, tag="h")
                        nc.scalar.activation(
                            ht[:, :hT], hps[fg][:, :hT], AF.Relu,
                            bias=fib_sb[:, fib_off + f: fib_off + f + 1])
                        hts.append(ht)
                    oaccs = [psum.tile([P, 512], F32, tag="mm", name=f"oacc{m}")
                             for m in range(EC)]
                    for fg in range(gsz):
                        f = g * gsz + fg
                        wt = wfo.tile([P, ED], F32R, tag="wfo")
                        nc.sync.dma_start(wt[:], fo_dram[ts(f, P), :])
                        for m in range(EC):
                            nc.tensor.matmul(
                                oaccs[m][:, :hT], wt[:, ts(m, P)],
                                hts[fg][:, :hT],
                                start=(fg == 0), stop=(fg == gsz - 1))
                    for m in range(EC):
                        dst = out_acc[:, m * T + h0: m * T + h0 + hT]
                        if g == 0:
                            nc.vector.tensor_scalar_add(
                                dst, oaccs[m][:, :hT], fob_sb[:, m:m + 1])
                        else:
                            nc.vector.tensor_tensor(
                                dst, dst, oaccs[m][:, :hT], op=ALU.add)

        # ====================== stacks (enc + dec) ======================
        with ExitStack() as sctx:
            sp = sctx.enter_context(tc.tile_pool(name="stack", bufs=1))
            wk = sctx.enter_context(tc.tile_pool(name="wk", bufs=2))
            wfi = sctx.enter_context(tc.tile_pool(name="wfi", bufs=2))
            wfo = sctx.enter_context(tc.tile_pool(name="wfo", bufs=2))
            hpool = sctx.enter_context(tc.tile_pool(name="h", bufs=9))
            expp = sctx.enter_context(tc.tile_pool(name="exp", bufs=2))
            gpool = sctx.enter_context(tc.tile_pool(name="g", bufs=2))
            vbp = sctx.enter_context(tc.tile_pool(name="vb", bufs=2))

            pe_sb = sp.tile([P, EC], F32, tag="pe")
            nc.sync.dma_start(pe_sb[:], pe_cols[:].transpose([1, 0]))
            encm_sb = sp.tile([P, SC], F32, tag="encm")
            nc.sync.dma_start(encm_sb[:], encm_d[:].transpose([1, 0]))
            decm_sb = sp.tile([P, TC * TS], F32, tag="decm")
            nc.sync.dma_start(
                decm_sb[:].rearrange("p (c q) -> p c q", c=TC),
                decm_d[:].transpose([1, 0, 2]))

            def embed(xt, emb_g, n_tc, T):
                for tcn in range(n_tc):
                    gt = gpool.tile([P, ED], F32, tag="emb")
                    nc.sync.dma_start(gt[:], emb_g[ts(tcn, P), :])
                    for c in range(EC):
                        tp = psum.tile([P, 512], F32, tag="mm")
                        nc.tensor.transpose(tp[:, :P], gt[:, ts(c, P)],
                                            ident[:])
                        nc.scalar.activation(
                            xt[:, c * T + tcn * P: c * T + (tcn + 1) * P],
                            tp[:, :P], AF.Identity, bias=pe_sb[:, c:c + 1])

            xt = sp.tile([P, EC * SS], F32R, tag="xt")
            embed(xt, semb_g, SC, SS)
            yt = sp.tile([P, EC * TS], F32R, tag="yt")
            embed(yt, temb_g, TC, TS)

            ga = sp.tile([P, 2 * EC], F32, tag="ga")
            bcol = sp.tile([P, 4 * EC + FC], F32, tag="bcol")
            vbrow = sp.tile([1, ED], F32, tag="vbrow")

            def load_layer_cols(pfx, i):
                nc.sync.dma_start(ga[:], W[pfx + "g"][i].transpose([1, 0]))
                nc.sync.dma_start(bcol[:, 0:EC],
                                  W[pfx + "Qb"][i].transpose([1, 0]))
                nc.sync.dma_start(bcol[:, EC:2 * EC],
                                  W[pfx + "Kb"][i].transpose([1, 0]))
                nc.sync.dma_start(bcol[:, 2 * EC:3 * EC],
                                  W[pfx + "Ob"][i].transpose([1, 0]))
                nc.sync.dma_start(bcol[:, 3 * EC:3 * EC + FC],
                                  W[pfx + "Fib"][i].transpose([1, 0]))
                fob = cols.tile([P, EC], F32, tag="fob")
                nc.sync.dma_start(fob[:], W[pfx + "Fob"][i].transpose([1, 0]))
                nc.sync.dma_start(vbrow[:], W[pfx + "Vbr"][i])
                vbB = [vbp.tile([P, 512], F32, tag="vbB", name=f"vbB{m}") for m in range(2)]
                for half in range(2):
                    nc.gpsimd.partition_broadcast(
                        vbB[half][:], vbrow[:, ts(half, 512)])
                return fob, vbB

            def layer(pfx, i, cur, T, res_tag, is_dec, x_enc=None):
                fob, vbB = load_layer_cols(pfx, i)
                qt = sp.tile([P, EC * SS], F32R, tag="qt")
                kt = sp.tile([P, EC * SS], F32R, tag="kt")
                v_sb = sp.tile([P, SC * VROW], F32R, tag="v")
                zt = sp.tile([P, EC * SS], F32R, tag="zt")
                work = sp.tile([P, EC * SS], F32R, tag="work")
                tt = sp.tile([P, EC * SS], F32R, tag="tt")

                def attn_block(q_src, kv_src, Tq, Tk, enc_mask, dec_mask):
                    proj(qt, q_src, Tq, W[pfx + "Q"][i], bcol, 0, wk)
                    proj(kt, kv_src, Tk, W[pfx + "K"][i], bcol, EC, wk)
                    v_project(v_sb, kv_src, Tk, W[pfx + "V"][i], vbB, wk)
                    attention(zt, qt, kt, v_sb, Tq, Tk, expp,
                              enc_mask=enc_mask, dec_mask=dec_mask)
                    proj(work, zt, Tq, W[pfx + "O"][i], bcol, 2 * EC, wk)

                if not is_dec:
                    attn_block(cur, cur, T, T, encm_sb, None)
                else:
                    attn_block(cur, cur, T, T, None, decm_sb)
                nc.vector.tensor_tensor(work[:, :EC * T], work[:, :EC * T],
                                        cur[:, :EC * T], op=ALU.add)
                ln(tt, work, T, zt, ga)
                if is_dec:
                    attn_block(tt, x_enc, T, SS, encm_sb, None)
                    nc.vector.tensor_tensor(work[:, :EC * T],
                                            work[:, :EC * T],
                                            tt[:, :EC * T], op=ALU.add)
                    ln(tt, work, T, zt, ga)
                ffn(work, tt, T, W[pfx + "Fi"][i], bcol, 3 * EC,
                    W[pfx + "Fo"][i], fob, wfi, wfo, hpool)
                nc.vector.tensor_tensor(work[:, :EC * T], work[:, :EC * T],
                                        tt[:, :EC * T], op=ALU.add)
                new = sp.tile([P, EC * T], F32R, tag=res_tag)
                ln(new, work, T, zt, ga)
                return new

            for i in range(L):
                xt = layer("e", i, xt, SS, "xt", False)
            for i in range(L):
                yt = layer("d", i, yt, TS, "yt", True, x_enc=xt)

            nc.sync.dma_start(
                y_bounce[:].transpose([1, 0, 2]),
                yt[:, :EC * TS].rearrange("p (c t) -> p c t", c=EC))

        # =========================== generator ==========================
        nc.gpsimd.collective_compute(
            "AllGather", ALU.bypass,
            replica_groups=[list(range(NCORES))],
            ins=[y_bounce.opt().bitcast(F32)],
            outs=[ag_out.opt().bitcast(F32)])

        with ExitStack() as gctx:
            gwp = gctx.enter_context(tc.tile_pool(name="gw", bufs=10))
            yp = gctx.enter_context(tc.tile_pool(name="yp", bufs=NG * EC))
            gbp = gctx.enter_context(tc.tile_pool(name="gb", bufs=2))
            stp = gctx.enter_context(tc.tile_pool(name="st", bufs=1))
            sc2 = gctx.enter_context(tc.tile_pool(name="sc2", bufs=4))

            z_dram = dram.tile([NG, P, VSL], F32, tag="zd")

            ytl = []
            for g in range(NG):
                bb, tcb = divmod(g, TC)
                rowt = []
                for e in range(EC):
                    t = yp.tile([P, P], F32R, tag="y", name=f"y{g}_{e}")
                    nc.sync.dma_start(t[:],
                                      ag_out[bb * EC + e, :, ts(tcb, P)])
                    rowt.append(t)
                ytl.append(rowt)

            mpos = stp.tile([P, NG], F32, tag="mpos")
            first_vc = [True] * NG
            voff = 0
            for vc, vw in enumerate(VCH):
                gwt = []
                for e in range(EC):
                    t = gwp.tile([P, 512], F32R, tag="gw", name=f"gw{vc}_{e}")
                    nc.sync.dma_start(t[:, :vw], gWs[ts(e, P), ds(voff, vw)])
                    gwt.append(t)
                gbrow = sc2.tile([1, 512], F32, tag="gbrow")
                nc.sync.dma_start(gbrow[:, :vw], gbs[:, voff:voff + vw])
                gbB = gbp.tile([P, 512], F32, tag="gbB")
                nc.gpsimd.partition_broadcast(gbB[:, :vw], gbrow[:, :vw])
                for g in range(NG):
                    acc = psum.tile([P, 512], F32, tag="mm")
                    for e in range(EC):
                        nc.tensor.matmul(acc[:, :vw], ytl[g][e][:],
                                         gwt[e][:, :vw],
                                         start=(e == 0), stop=(e == EC - 1))
                    zc = sc2.tile([P, 512], F32, tag="zc")
                    nc.vector.tensor_tensor(zc[:, :vw], acc[:, :vw],
                                            gbB[:, :vw], op=ALU.add)
                    nc.sync.dma_start(z_dram[g, :, ds(voff, vw)], zc[:, :vw])
                    rm = sc2.tile([P, 1], F32, tag="rm")
                    nc.vector.tensor_reduce(rm[:], zc[:, :vw], axis=AX.X,
                                            op=ALU.max)
                    if first_vc[g]:
                        nc.vector.tensor_copy(mpos[:, g:g + 1], rm[:])
                        first_vc[g] = False
                    else:
                        nc.vector.tensor_tensor(mpos[:, g:g + 1],
                                                mpos[:, g:g + 1], rm[:],
                                                op=ALU.max)
                voff += vw

            nc.sync.dma_start(m_bounce[:], mpos[:])
            nc.gpsimd.collective_compute(
                "AllReduce", ALU.max,
                replica_groups=[list(range(NCORES))],
                ins=[m_bounce.opt()], outs=[m_red.opt()])
            mg = stp.tile([P, NG], F32, tag="mg")
            nc.sync.dma_start(mg[:], m_red[:])
            mneg = stp.tile([P, NG], F32, tag="mneg")
            nc.scalar.mul(mneg[:], mg[:], -1.0)

            parts = stp.tile([P, NG * len(VCH)], F32, tag="parts")
            for g in range(NG):
                voff = 0
                for vc, vw in enumerate(VCH):
                    zc = sc2.tile([P, 512], F32, tag="zc")
                    nc.sync.dma_start(zc[:, :vw], z_dram[g, :, ds(voff, vw)])
                    scr = sc2.tile([P, 512], F32, tag="scr")
                    nc.scalar.activation(
                        scr[:, :vw], zc[:, :vw], AF.Exp,
                        bias=mneg[:, g:g + 1],
                        accum_out=parts[:, g * len(VCH) + vc:
                                        g * len(VCH) + vc + 1])
                    voff += vw
            sloc = stp.tile([P, NG], F32, tag="sloc")
            nc.vector.tensor_reduce(
                sloc[:], parts[:].rearrange("p (g v) -> p g v", g=NG),
                axis=AX.X, op=ALU.add)
            nc.sync.dma_start(s_bounce[:], sloc[:])
            nc.gpsimd.collective_compute(
                "AllReduce", ALU.add,
                replica_groups=[list(range(NCORES))],
                ins=[s_bounce.opt()], outs=[s_red.opt()])
            sg = stp.tile([P, NG], F32, tag="sg")
            nc.sync.dma_start(sg[:], s_red[:])

            lg = stp.tile([P, NG], F32, tag="lg")
            nc.scalar.activation(lg[:], sg[:], AF.Ln)
            shift = stp.tile([P, NG], F32, tag="shift")
            nc.vector.tensor_tensor(shift[:], mneg[:], lg[:],
                                    op=ALU.subtract)
            for g in range(NG):
                voff = 0
                for vw in VCH:
                    zc = sc2.tile([P, 512], F32, tag="zc")
                    nc.sync.dma_start(zc[:, :vw], z_dram[g, :, ds(voff, vw)])
                    nc.scalar.activation(zc[:, :vw], zc[:, :vw],
                                         AF.Identity, bias=shift[:, g:g + 1])
                    nc.sync.dma_start(out_d[ts(g, P), ds(voff, vw)],
                                      zc[:, :vw])
                    voff += vw

    nc.compile()
    return nc


# ------------------------------------------------------------------- host --
def _pe_rows(n, ed=ED):
    pos = np.arange(n, dtype=np.float32)[:, None]
    div = np.exp(np.arange(0, ed, 2, dtype=np.float32)
                 * (-np.log(10000.0) / ed))
    pe = np.zeros((n, ed), np.float32)
    pe[:, 0::2] = np.sin(pos * div)
    pe[:, 1::2] = np.cos(pos * div)
    return pe


_NC_CACHE = None


def kernel(**inputs):
    global _NC_CACHE, LAST_RESULTS
    inp = {k: np.asarray(v) for k, v in inputs.items()}
    src, tgt = inp["src"], inp["tgt"]

    if _NC_CACHE is None:
        _NC_CACHE = build()
    nc = _NC_CACHE

    pe = _pe_rows(max(BS, 8))

    def f32(x):
        return np.ascontiguousarray(x, dtype=np.float32)

    shared = {}
    for p in ("e", "d"):
        shared[p + "Qp"] = f32(inp[p + "WQ"][:, :, HEAD_PERM])
        shared[p + "Kp"] = f32(inp[p + "WK"][:, :, HEAD_PERM])
        shared[p + "Vp"] = f32(inp[p + "WV"][:, :, HEAD_PERM])
        shared[p + "O"] = f32(inp[p + "WO"])
        shared[p + "Fi"] = f32(inp[p + "Fi"])
        shared[p + "Fo"] = f32(inp[p + "Fo"])
        shared[p + "Qbp"] = f32(inp[p + "WQb"][:, HEAD_PERM]).reshape(L, EC, P)
        shared[p + "Kbp"] = f32(inp[p + "WKb"][:, HEAD_PERM]).reshape(L, EC, P)
        shared[p + "Vbr"] = f32(inp[p + "WVb"][:, HEAD_PERM]).reshape(L, 1, ED)
        shared[p + "Ob"] = f32(inp[p + "WOb"]).reshape(L, EC, P)
        shared[p + "Fib"] = f32(inp[p + "Fib"]).reshape(L, FC, P)
        shared[p + "Fob"] = f32(inp[p + "Fob"]).reshape(L, EC, P)
        shared[p + "g"] = f32(np.concatenate(
            [inp[p + "na"].reshape(L, EC, P),
             inp[p + "nb"].reshape(L, EC, P)], axis=1))

    causal_kq = np.tril(np.ones((TS, TS), np.float32)).T  # [k, q]: keep k<=q

    in_maps = []
    for c in range(NCORES):
        b = c % BS
        m = dict(shared)
        m["semb_g"] = f32(inp["src_emb"][src[b]])
        m["temb_g"] = f32(inp["tgt_emb"][tgt[b]])
        m["pe_cols"] = f32(pe[b].reshape(EC, P))
        m["encm"] = f32(np.where(src[b] != PAD, 0.0, NEG).reshape(SC, P))
        keep = (tgt[b] != PAD).astype(np.float32)[:, None] * causal_kq
        m["decm8"] = f32(np.where(keep > 0, 0.0, -1e9).reshape(TC, P, TS))
        m["gWs"] = f32(inp["gW"][:, c * VSL:(c + 1) * VSL])
        m["gbs"] = f32(inp["gb"][c * VSL:(c + 1) * VSL].reshape(1, VSL))
        in_maps.append(m)

    res = run_bass_kernel_spmd(nc, in_maps, list(range(NCORES)))
    LAST_RESULTS = res
    out = np.concatenate(
        [res.results[c]["out"].reshape(BS, TS, VSL) for c in range(NCORES)],
        axis=-1)
    return out.astype(np.float32)


# revision 12
# speedup vs baseline: 1.0043x; 1.0043x over previous
"""Trainium2 Bass kernel for nn_Encoder_Decoder_60576218743336.

Strategy (8 NeuronCores, SPMD):
- Transformer stacks: data-parallel over batch (4). Core c handles batch c%4
  (cores 4-7 duplicate batches 0-3 so the whole chip participates in the
  vocab-parallel generator afterwards).
- Activations are feature-major XT[feature, token] folded into SBUF tiles
  [128, 8*T]; every projection uses the stored weights [in, out] directly as
  the PE stationary operand (out = W.T @ XT). fp32r matmuls (TF32-like,
  full PE rate at free-dim >= 256).
- Attention: WQ/WK/WV columns permuted host-side to head-major so per-head
  slices are contiguous; scores computed in ST[k, q] layout so the softmax
  normalizer is a ones-column appended to V (no transposes; exp without
  max-subtraction -- exact vs the reference semantics, validated in numpy).
- LayerNorm over the feature (partition) axis via ones-matmul column sums,
  unbiased std, (std + eps) as in the reference.
- Generator: vocab-parallel over all 8 cores: AllGather y, each core computes
  logits for its 4000-wide vocab slice, distributed log-softmax via
  AllReduce(min of -max) + AllReduce(add) of per-token stats.

kernel(**inputs) takes the FULL inputs (named as reference.setup_inputs())
and returns the FULL (4, 256, 32000) float32 output.
"""

from contextlib import ExitStack

import numpy as np

import concourse.bacc as bacc
import concourse.bass as bass
import concourse.mybir as mybir
import concourse.tile as tile
from concourse.bass import ds, ts
from concourse.bass_utils import run_bass_kernel_spmd
from concourse.masks import make_identity

P = 128
L, ED, NH, QD, VD, FF, VS, VT = 4, 1024, 16, 64, 64, 4096, 32000, 32000
BS, SS, TS, PAD = 4, 512, 256, 0
EPS = 1e-6
SCALE = float(1.0 / np.sqrt(QD))
NEG = np.float32(-1e9 * SCALE)
EC = ED // P            # 8 feature chunks
FC = FF // P            # 32 ff chunks
SC = SS // P            # 4 src token chunks
TC = TS // P            # 2 tgt token chunks
NCORES = 8
VSL = VT // NCORES      # 4000 vocab columns per core
VCH = [512] * 7 + [VSL - 512 * 7]
NG = BS * TC            # 8 token-chunk groups in the generator

F32 = mybir.dt.float32
F32R = mybir.dt.float32r
I32 = mybir.dt.int32
AF = mybir.ActivationFunctionType
ALU = mybir.AluOpType
AX = mybir.AxisListType

HEAD_PERM = np.array([d * NH + h for h in range(NH) for d in range(QD)])
VROW = NH * (VD + 1)    # token-major V row: 16 heads x (64 vals + 1.0)

LAST_RESULTS = None


def build():
    nc = bacc.Bacc("TRN2", target_bir_lowering=False, debug=False,
                   num_devices=NCORES)

    def din(name, shape, dtype=F32R):
        return nc.dram_tensor(name, shape, dtype, kind="ExternalInput").ap()

    semb_g = din("semb_g", [SS, ED], F32)
    temb_g = din("temb_g", [TS, ED], F32)
    pe_cols = din("pe_cols", [EC, P], F32)
    encm_d = din("encm", [SC, P], F32)
    decm_d = din("decm8", [TC, P, TS], F32)
    gWs = din("gWs", [ED, VSL], F32R)
    gbs = din("gbs", [1, VSL], F32R)

    W = {}
    for pfx in ("e", "d"):
        W[pfx + "Q"] = din(pfx + "Qp", [L, ED, ED])
        W[pfx + "K"] = din(pfx + "Kp", [L, ED, ED])
        W[pfx + "V"] = din(pfx + "Vp", [L, ED, ED])
        W[pfx + "O"] = din(pfx + "O", [L, ED, ED])
        W[pfx + "Fi"] = din(pfx + "Fi", [L, ED, FF])
        W[pfx + "Fo"] = din(pfx + "Fo", [L, FF, ED])
        W[pfx + "Qb"] = din(pfx + "Qbp", [L, EC, P], F32)
        W[pfx + "Kb"] = din(pfx + "Kbp", [L, EC, P], F32)
        W[pfx + "Vbr"] = din(pfx + "Vbr", [L, 1, ED], F32R)
        W[pfx + "Ob"] = din(pfx + "Ob", [L, EC, P], F32)
        W[pfx + "Fib"] = din(pfx + "Fib", [L, FC, P], F32)
        W[pfx + "Fob"] = din(pfx + "Fob", [L, EC, P], F32)
        W[pfx + "g"] = din(pfx + "g", [L, 2 * EC, P], F32)

    out_d = nc.dram_tensor("out", [BS * TS, VSL], F32,
                           kind="ExternalOutput").ap()

    with ExitStack() as ctx:
        tc = ctx.enter_context(tile.TileContext(nc))

        const = ctx.enter_context(tc.tile_pool(name="const", bufs=1))
        psum = ctx.enter_context(tc.tile_pool(name="psum", bufs=8,
                                              space="PSUM"))
        rows = ctx.enter_context(tc.tile_pool(name="rows", bufs=1))
        arow = ctx.enter_context(tc.tile_pool(name="arow", bufs=2))
        cols = ctx.enter_context(tc.tile_pool(name="cols", bufs=2))
        dram = ctx.enter_context(tc.tile_pool(name="dram", bufs=1,
                                              space="DRAM"))

        onesf = const.tile([P, 1], F32, tag="onesf")
        nc.any.memset(onesf[:], 1.0)
        ones_col = const.tile([P, 1], F32R, tag="ones_col")
        nc.vector.tensor_copy(ones_col[:], onesf[:])
        onesrf = const.tile([1, P], F32, tag="onesrf")
        nc.any.memset(onesrf[:], 1.0)
        ones_row = const.tile([1, P], F32R, tag="ones_row")
        nc.vector.tensor_copy(ones_row[:], onesrf[:])
        ident = const.tile([P, P], F32, tag="ident")
        make_identity(nc, ident[:])
        eps_row = const.tile([1, 1], F32, tag="eps_row")
        nc.any.memset(eps_row[:], EPS)

        y_bounce = dram.tile([EC, P, TS], F32R, tag="yb")
        ag_out = dram.tile([NCORES * EC, P, TS], F32R, tag="ag")
        m_bounce = dram.tile([P, NG], F32, tag="mb")
        m_red = dram.tile([P, NG], F32, tag="mr")
        s_bounce = dram.tile([P, NG], F32, tag="sb")
        s_red = dram.tile([P, NG], F32, tag="sr")

        # ============================ helpers ===========================
        def proj(out_sb, x_sb, T, w_dram, bias_sb, bias_off, wk):
            """out[f, t] = W.T @ x + b  (feature-major, EC->EC chunks)."""
            accs = [psum.tile([P, 512], F32, tag="mm", name=f"pacc{m}") for m in range(EC)]
            for e in range(EC):
                wt = wk.tile([P, ED], F32R, tag="w")
                nc.sync.dma_start(wt[:], w_dram[ts(e, P), :])
                for m in range(EC):
                    nc.tensor.matmul(
                        accs[m][:, :T], wt[:, ts(m, P)],
                        x_sb[:, e * T:(e + 1) * T],
                        start=(e == 0), stop=(e == EC - 1))
            for m in range(EC):
                nc.vector.tensor_scalar_add(
                    out_sb[:, m * T:(m + 1) * T], accs[m][:, :T],
                    bias_sb[:, bias_off + m: bias_off + m + 1])

        def v_project(v_sb, x_sb, T, w_dram, vbB, wk):
            """token-major V with per-head ones column appended."""
            n_tc = T // P
            accs = [psum.tile([P, 512], F32, tag="mm", name=f"vacc{m}")
                    for m in range(n_tc * 2)]
            for e in range(EC):
                wt = wk.tile([P, ED], F32R, tag="w")
                nc.sync.dma_start(wt[:], w_dram[ts(e, P), :])
                for kc in range(n_tc):
                    for half in range(2):
                        nc.tensor.matmul(
                            accs[kc * 2 + half][:],
                            x_sb[:, e * T + kc * P: e * T + (kc + 1) * P],
                            wt[:, ts(half, 512)],
                            start=(e == 0), stop=(e == EC - 1))
            for kc in range(n_tc):
                for half in range(2):
                    base = kc * VROW + half * 8 * (VD + 1)
                    dst = v_sb[:, base: base + 8 * (VD + 1)].rearrange(
                        "p (h d) -> p h d", h=8)[:, :, :VD]
                    src3 = accs[kc * 2 + half][:].rearrange(
                        "p (h d) -> p h d", h=8)
                    vb3 = vbB[half][:].rearrange("p (h d) -> p h d", h=8)
                    nc.vector.tensor_tensor(dst, src3, vb3, op=ALU.add)
                ones_dst = v_sb[:, kc * VROW:(kc + 1) * VROW].rearrange(
                    "p (h d) -> p h d", h=NH)[:, :, VD:]
                nc.vector.tensor_copy(
                    ones_dst,
                    ones_col[:].unsqueeze(1).broadcast_to([P, NH, 1]))

        def attention(zt, qt, kt, v_sb, Tq, Tk, expp,
                      enc_mask=None, dec_mask=None):
            n_kc = Tk // P
            for h in range(NH):
                qp, qo = (h % 2) * 64, (h // 2) * Tq
                e_t = expp.tile([P, 4 * 512], F32R, tag="exp")
                for kc in range(n_kc):
                    st = psum.tile([P, 512], F32, tag="mm")
                    nc.tensor.matmul(
                        st[:, :Tq],
                        kt[qp:qp + 64,
                           (h // 2) * Tk + kc * P:(h // 2) * Tk + (kc + 1) * P],
                        qt[qp:qp + 64, qo:qo + Tq],
                        start=True, stop=True)
                    if dec_mask is not None:
                        nc.vector.tensor_tensor(
                            st[:, :Tq], st[:, :Tq],
                            dec_mask[:, kc * Tq:(kc + 1) * Tq], op=ALU.add)
                        nc.scalar.activation(
                            e_t[:, kc * Tq:(kc + 1) * Tq], st[:, :Tq],
                            AF.Exp, scale=SCALE)
                    else:
                        nc.scalar.activation(
                            e_t[:, kc * Tq:(kc + 1) * Tq], st[:, :Tq],
                            AF.Exp, bias=enc_mask[:, kc:kc + 1], scale=SCALE)
                zp = psum.tile([P, 512], F32, tag="mm")
                for kc in range(n_kc):
                    nc.tensor.matmul(
                        zp[:VD + 1, :Tq],
                        v_sb[:, kc * VROW + h * (VD + 1):
                             kc * VROW + (h + 1) * (VD + 1)],
                        e_t[:, kc * Tq:(kc + 1) * Tq],
                        start=(kc == 0), stop=(kc == n_kc - 1))
                rcp = arow.tile([1, 512], F32R, tag="rcp")
                with nc.allow_low_precision(reason="f32r recip, fp32-width"):
                    nc.vector.reciprocal(rcp[:, :Tq], zp[VD:VD + 1, :Tq])
                rb = psum.tile([P, 512], F32, tag="mm")
                nc.tensor.matmul(rb[:VD, :Tq], ones_row[:, :VD],
                                 rcp[:, :Tq], start=True, stop=True)
                rbs = arow.tile([VD, 512], F32, tag="rbs")
                nc.scalar.copy(rbs[:, :Tq], rb[:VD, :Tq])
                nc.vector.tensor_tensor(
                    zt[qp:qp + 64, qo:qo + Tq], zp[:VD, :Tq],
                    rbs[:, :Tq], op=ALU.mult)

        def ln(out_sb, a_sb, T, sq_sb, g_sb):
            """LN over 1024 features, per token; a/out folded [128, 8*T]."""
            nf = ED
            nc.vector.tensor_tensor(sq_sb[:, :EC * T], a_sb[:, :EC * T],
                                    a_sb[:, :EC * T], op=ALU.mult)
            psS = psum.tile([P, 512], F32, tag="mm")
            psQ = psum.tile([P, 512], F32, tag="mm")
            for c in range(EC):
                nc.tensor.matmul(psS[:1, :T], ones_col[:],
                                 a_sb[:, c * T:(c + 1) * T],
                                 start=(c == 0), stop=(c == EC - 1))
            for c in range(EC):
                nc.tensor.matmul(psQ[:1, :T], ones_col[:],
                                 sq_sb[:, c * T:(c + 1) * T],
                                 start=(c == 0), stop=(c == EC - 1))
            mu = rows.tile([1, 512], F32R, tag="mu")
            nc.scalar.mul(mu[:, :T], psS[:1, :T], 1.0 / nf)
            sA = rows.tile([1, 512], F32, tag="sA")
            nc.vector.tensor_tensor(sA[:, :T], mu[:, :T], mu[:, :T],
                                    op=ALU.mult)
            nc.scalar.mul(sA[:, :T], sA[:, :T], float(nf) / (nf - 1))
            sB = rows.tile([1, 512], F32, tag="sB")
            nc.scalar.mul(sB[:, :T], psQ[:1, :T], 1.0 / (nf - 1))
            nc.vector.tensor_tensor(sB[:, :T], sB[:, :T], sA[:, :T],
                                    op=ALU.subtract)
            nc.scalar.activation(sB[:, :T], sB[:, :T], AF.Sqrt)
            nc.scalar.activation(sB[:, :T], sB[:, :T], AF.Identity,
                                 bias=eps_row[:])
            rstd = rows.tile([1, 512], F32R, tag="rstd")
            with nc.allow_low_precision(reason="f32r recip, fp32-width"):
                nc.vector.reciprocal(rstd[:, :T], sB[:, :T])
            muB = psum.tile([P, 512], F32, tag="mm")
            rsB = psum.tile([P, 512], F32, tag="mm")
            nc.tensor.matmul(muB[:, :T], ones_row[:], mu[:, :T],
                             start=True, stop=True)
            nc.tensor.matmul(rsB[:, :T], ones_row[:], rstd[:, :T],
                             start=True, stop=True)
            a3 = a_sb[:, :EC * T].rearrange("p (c t) -> p c t", c=EC)
            o3 = out_sb[:, :EC * T].rearrange("p (c t) -> p c t", c=EC)
            muB3 = muB[:, :T].unsqueeze(1).broadcast_to([P, EC, T])
            rsB3 = rsB[:, :T].unsqueeze(1).broadcast_to([P, EC, T])
            nc.vector.tensor_tensor(o3, a3, muB3, op=ALU.subtract)
            nc.vector.tensor_tensor(o3, o3, rsB3, op=ALU.mult)
            for c in range(EC):
                nc.scalar.activation(
                    out_sb[:, c * T:(c + 1) * T], out_sb[:, c * T:(c + 1) * T],
                    AF.Identity,
                    bias=g_sb[:, EC + c: EC + c + 1],
                    scale=g_sb[:, c:c + 1])

        def ffn(out_acc, t_sb, T, fi_dram, fib_sb, fib_off, fo_dram, fob_sb,
                wfi, wfo, hpool):
            halves = [(0, T)] if T <= 256 else [(0, 256), (256, 256)]
            gsz = 8
            for g in range(FC // gsz):
                for h0, hT in halves:
                    hps = [psum.tile([P, 512], F32, tag="mm", name=f"hp{m}")
                           for m in range(gsz)]
                    for e in range(EC):
                        wt = wfi.tile([P, gsz * P], F32R, tag="wfi")
                        nc.sync.dma_start(
                            wt[:], fi_dram[ts(e, P), ds(g * gsz * P, gsz * P)])
                        for fg in range(gsz):
                            nc.tensor.matmul(
                                hps[fg][:, :hT], wt[:, ts(fg, P)],
                                t_sb[:, e * T + h0: e * T + h0 + hT],
                                start=(e == 0), stop=(e == EC - 1))
                    hts = []
                    for fg in range(gsz):
                        f = g * gsz + fg
                        ht = hpool.tile([P, 256], F32R, tag="h")
                        nc.scalar.activation(
                            ht[:, :hT], hps[fg][:, :hT], AF.Relu,
                            bias=fib_sb[:, fib_off + f: fib_off + f + 1])
                        hts.append(ht)
                    oaccs = [psum.tile([P, 512], F32, tag="mm", name=f"oacc{m}")
                             for m in range(EC)]
                    for fg in range(gsz):
                        f = g * gsz + fg
                        wt = wfo.tile([P, ED], F32R, tag="wfo")
                        nc.sync.dma_start(wt[:], fo_dram[ts(f, P), :])
                        for m in range(EC):
                            nc.tensor.matmul(
                                oaccs[m][:, :hT], wt[:, ts(m, P)],
                                hts[fg][:, :hT],
                                start=(fg == 0), stop=(fg == gsz - 1))
                    for m in range(EC):
                        dst = out_acc[:, m * T + h0: m * T + h0 + hT]
                        if g == 0:
                            nc.vector.tensor_scalar_add(
                                dst, oaccs[m][:, :hT], fob_sb[:, m:m + 1])
                        else:
                            nc.vector.tensor_tensor(
                                dst, dst, oaccs[m][:, :hT], op=ALU.add)

        # ====================== stacks (enc + dec) ======================
        with ExitStack() as sctx:
            sp = sctx.enter_context(tc.tile_pool(name="stack", bufs=1))
            wk = sctx.enter_context(tc.tile_pool(name="wk", bufs=2))
            wfi = sctx.enter_context(tc.tile_pool(name="wfi", bufs=2))
            wfo = sctx.enter_context(tc.tile_pool(name="wfo", bufs=2))
            hpool = sctx.enter_context(tc.tile_pool(name="h", bufs=9))
            expp = sctx.enter_context(tc.tile_pool(name="exp", bufs=2))
            gpool = sctx.enter_context(tc.tile_pool(name="g", bufs=2))
            vbp = sctx.enter_context(tc.tile_pool(name="vb", bufs=2))

            pe_sb = sp.tile([P, EC], F32, tag="pe")
            nc.sync.dma_start(pe_sb[:], pe_cols[:].transpose([1, 0]))
            encm_sb = sp.tile([P, SC], F32, tag="encm")
            nc.sync.dma_start(encm_sb[:], encm_d[:].transpose([1, 0]))
            decm_sb = sp.tile([P, TC * TS], F32, tag="decm")
            nc.sync.dma_start(
                decm_sb[:].rearrange("p (c q) -> p c q", c=TC),
                decm_d[:].transpose([1, 0, 2]))

            def embed(xt, emb_g, n_tc, T):
                for tcn in range(n_tc):
                    gt = gpool.tile([P, ED], F32, tag="emb")
                    nc.sync.dma_start(gt[:], emb_g[ts(tcn, P), :])
                    for c in range(EC):
                        tp = psum.tile([P, 512], F32, tag="mm")
                        nc.tensor.transpose(tp[:, :P], gt[:, ts(c, P)],
                                            ident[:])
                        nc.scalar.activation(
                            xt[:, c * T + tcn * P: c * T + (tcn + 1) * P],
                            tp[:, :P], AF.Identity, bias=pe_sb[:, c:c + 1])

            xt = sp.tile([P, EC * SS], F32R, tag="xt")
            embed(xt, semb_g, SC, SS)
            yt = sp.tile([P, EC * TS], F32R, tag="yt")
            embed(yt, temb_g, TC, TS)

            ga = sp.tile([P, 2 * EC], F32, tag="ga")
            bcol = sp.tile([P, 4 * EC + FC], F32, tag="bcol")
            vbrow = sp.tile([1, ED], F32R, tag="vbrow")

            def load_layer_cols(pfx, i):
                nc.sync.dma_start(ga[:], W[pfx + "g"][i].transpose([1, 0]))
                nc.sync.dma_start(bcol[:, 0:EC],
                                  W[pfx + "Qb"][i].transpose([1, 0]))
                nc.sync.dma_start(bcol[:, EC:2 * EC],
                                  W[pfx + "Kb"][i].transpose([1, 0]))
                nc.sync.dma_start(bcol[:, 2 * EC:3 * EC],
                                  W[pfx + "Ob"][i].transpose([1, 0]))
                nc.sync.dma_start(bcol[:, 3 * EC:3 * EC + FC],
                                  W[pfx + "Fib"][i].transpose([1, 0]))
                fob = cols.tile([P, EC], F32, tag="fob")
                nc.sync.dma_start(fob[:], W[pfx + "Fob"][i].transpose([1, 0]))
                nc.sync.dma_start(vbrow[:], W[pfx + "Vbr"][i])
                vbB = [vbp.tile([P, 512], F32, tag="vbB", name=f"vbB{m}") for m in range(2)]
                for half in range(2):
                    vps = psum.tile([P, 512], F32, tag="mm", name="vbps")
                    nc.tensor.matmul(vps[:], ones_row[:],
                                     vbrow[:, ts(half, 512)],
                                     start=True, stop=True)
                    nc.scalar.copy(vbB[half][:], vps[:])
                return fob, vbB

            def layer(pfx, i, cur, T, res_tag, is_dec, x_enc=None):
                fob, vbB = load_layer_cols(pfx, i)
                qt = sp.tile([P, EC * SS], F32R, tag="qt")
                kt = sp.tile([P, EC * SS], F32R, tag="kt")
                v_sb = sp.tile([P, SC * VROW], F32R, tag="v")
                zt = sp.tile([P, EC * SS], F32R, tag="zt")
                work = sp.tile([P, EC * SS], F32R, tag="work")
                tt = sp.tile([P, EC * SS], F32R, tag="tt")

                def attn_block(q_src, kv_src, Tq, Tk, enc_mask, dec_mask):
                    proj(qt, q_src, Tq, W[pfx + "Q"][i], bcol, 0, wk)
                    proj(kt, kv_src, Tk, W[pfx + "K"][i], bcol, EC, wk)
                    v_project(v_sb, kv_src, Tk, W[pfx + "V"][i], vbB, wk)
                    attention(zt, qt, kt, v_sb, Tq, Tk, expp,
                              enc_mask=enc_mask, dec_mask=dec_mask)
                    proj(work, zt, Tq, W[pfx + "O"][i], bcol, 2 * EC, wk)

                if not is_dec:
                    attn_block(cur, cur, T, T, encm_sb, None)
                else:
                    attn_block(cur, cur, T, T, None, decm_sb)
                nc.vector.tensor_tensor(work[:, :EC * T], work[:, :EC * T],
                                        cur[:, :EC * T], op=ALU.add)
                ln(tt, work, T, zt, ga)
                if is_dec:
                    attn_block(tt, x_enc, T, SS, encm_sb, None)
                    nc.vector.tensor_tensor(work[:, :EC * T],
                                            work[:, :EC * T],
                                            tt[:, :EC * T], op=ALU.add)
                    ln(tt, work, T, zt, ga)
                ffn(work, tt, T, W[pfx + "Fi"][i], bcol, 3 * EC,
                    W[pfx + "Fo"][i], fob, wfi, wfo, hpool)
                nc.vector.tensor_tensor(work[:, :EC * T], work[:, :EC * T],
                                        tt[:, :EC * T], op=ALU.add)
                new = sp.tile([P, EC * T], F32R, tag=res_tag)
                ln(new, work, T, zt, ga)
                return new

            for i in range(L):
                xt = layer("e", i, xt, SS, "xt", False)
            for i in range(L):
                yt = layer("d", i, yt, TS, "yt", True, x_enc=xt)

            nc.sync.dma_start(
                y_bounce[:].transpose([1, 0, 2]),
                yt[:, :EC * TS].rearrange("p (c t) -> p c t", c=EC))

        # =========================== generator ==========================
        nc.gpsimd.collective_compute(
            "AllGather", ALU.bypass,
            replica_groups=[list(range(NCORES))],
            ins=[y_bounce.opt().bitcast(F32)],
            outs=[ag_out.opt().bitcast(F32)])

        with ExitStack() as gctx:
            gwp = gctx.enter_context(tc.tile_pool(name="gw", bufs=10))
            yp = gctx.enter_context(tc.tile_pool(name="yp", bufs=NG * EC))
            gbp = gctx.enter_context(tc.tile_pool(name="gb", bufs=2))
            stp = gctx.enter_context(tc.tile_pool(name="st", bufs=1))
            sc2 = gctx.enter_context(tc.tile_pool(name="sc2", bufs=4))

            z_dram = dram.tile([NG, P, VSL], F32, tag="zd")

            ytl = []
            for g in range(NG):
                bb, tcb = divmod(g, TC)
                rowt = []
                for e in range(EC):
                    t = yp.tile([P, P], F32R, tag="y", name=f"y{g}_{e}")
                    nc.sync.dma_start(t[:],
                                      ag_out[bb * EC + e, :, ts(tcb, P)])
                    rowt.append(t)
                ytl.append(rowt)

            mpos = stp.tile([P, NG], F32, tag="mpos")
            first_vc = [True] * NG
            voff = 0
            for vc, vw in enumerate(VCH):
                gwt = []
                for e in range(EC):
                    t = gwp.tile([P, 512], F32R, tag="gw", name=f"gw{vc}_{e}")
                    nc.sync.dma_start(t[:, :vw], gWs[ts(e, P), ds(voff, vw)])
                    gwt.append(t)
                gbrow = sc2.tile([1, 512], F32R, tag="gbrow")
                nc.sync.dma_start(gbrow[:, :vw], gbs[:, voff:voff + vw])
                gbB = gbp.tile([P, 512], F32, tag="gbB")
                gps = psum.tile([P, 512], F32, tag="mm", name="gbps")
                nc.tensor.matmul(gps[:, :vw], ones_row[:], gbrow[:, :vw],
                                 start=True, stop=True)
                nc.scalar.copy(gbB[:, :vw], gps[:, :vw])
                for g in range(NG):
                    acc = psum.tile([P, 512], F32, tag="mm")
                    for e in range(EC):
                        nc.tensor.matmul(acc[:, :vw], ytl[g][e][:],
                                         gwt[e][:, :vw],
                                         start=(e == 0), stop=(e == EC - 1))
                    zc = sc2.tile([P, 512], F32, tag="zc")
                    nc.vector.tensor_tensor(zc[:, :vw], acc[:, :vw],
                                            gbB[:, :vw], op=ALU.add)
                    nc.sync.dma_start(z_dram[g, :, ds(voff, vw)], zc[:, :vw])
                    rm = sc2.tile([P, 1], F32, tag="rm")
                    nc.vector.tensor_reduce(rm[:], zc[:, :vw], axis=AX.X,
                                            op=ALU.max)
                    if first_vc[g]:
                        nc.vector.tensor_copy(mpos[:, g:g + 1], rm[:])
                        first_vc[g] = False
                    else:
                        nc.vector.tensor_tensor(mpos[:, g:g + 1],
                                                mpos[:, g:g + 1], rm[:],
                                                op=ALU.max)
                voff += vw

            nc.sync.dma_start(m_bounce[:], mpos[:])
            nc.gpsimd.collective_compute(
                "AllReduce", ALU.max,
                replica_groups=[list(range(NCORES))],
                ins=[m_bounce.opt()], outs=[m_red.opt()])
            mg = stp.tile([P, NG], F32, tag="mg")
            nc.sync.dma_start(mg[:], m_red[:])
            mneg = stp.tile([P, NG], F32, tag="mneg")
            nc.scalar.mul(mneg[:], mg[:], -1.0)

            parts = stp.tile([P, NG * len(VCH)], F32, tag="parts")
            for g in range(NG):
                voff = 0
                for vc, vw in enumerate(VCH):
                    zc = sc2.tile([P, 512], F32, tag="zc")
                    nc.sync.dma_start(zc[:, :vw], z_dram[g, :, ds(voff, vw)])
                    scr = sc2.tile([P, 512], F32, tag="scr")
                    nc.scalar.activation(
                        scr[:, :vw], zc[:, :vw], AF.Exp,
                        bias=mneg[:, g:g + 1],
                        accum_out=parts[:, g * len(VCH) + vc:
                                        g * len(VCH) + vc + 1])
                    voff += vw
            sloc = stp.tile([P, NG], F32, tag="sloc")
            nc.vector.tensor_reduce(
                sloc[:], parts[:].rearrange("p (g v) -> p g v", g=NG),
                axis=AX.X, op=ALU.add)
            nc.sync.dma_start(s_bounce[:], sloc[:])
            nc.gpsimd.collective_compute(
                "AllReduce", ALU.add,
                replica_groups=[list(range(NCORES))],
                ins=[s_bounce.opt()], outs=[s_red.opt()])
            sg = stp.tile([P, NG], F32, tag="sg")
            nc.sync.dma_start(sg[:], s_red[:])

            lg = stp.tile([P, NG], F32, tag="lg")
            nc.scalar.activation(lg[:], sg[:], AF.Ln)
            shift = stp.tile([P, NG], F32, tag="shift")
            nc.vector.tensor_tensor(shift[:], mneg[:], lg[:],
                                    op=ALU.subtract)
            for g in range(NG):
                voff = 0
                for vw in VCH:
                    zc = sc2.tile([P, 512], F32, tag="zc")
                    nc.sync.dma_start(zc[:, :vw], z_dram[g, :, ds(voff, vw)])
                    nc.scalar.activation(zc[:, :vw], zc[:, :vw],
                                         AF.Identity, bias=shift[:, g:g + 1])
                    nc.sync.dma_start(out_d[ts(g, P), ds(voff, vw)],
                                      zc[:, :vw])
                    voff += vw

    nc.compile()
    return nc


# ------------------------------------------------------------------- host --
def _pe_rows(n, ed=ED):
    pos = np.arange(n, dtype=np.float32)[:, None]
    div = np.exp(np.arange(0, ed, 2, dtype=np.float32)
                 * (-np.log(10000.0) / ed))
    pe = np.zeros((n, ed), np.float32)
    pe[:, 0::2] = np.sin(pos * div)
    pe[:, 1::2] = np.cos(pos * div)
    return pe


_NC_CACHE = None


def kernel(**inputs):
    global _NC_CACHE, LAST_RESULTS
    inp = {k: np.asarray(v) for k, v in inputs.items()}
    src, tgt = inp["src"], inp["tgt"]

    if _NC_CACHE is None:
        _NC_CACHE = build()
    nc = _NC_CACHE

    pe = _pe_rows(max(BS, 8))

    def f32(x):
        return np.ascontiguousarray(x, dtype=np.float32)

    shared = {}
    for p in ("e", "d"):
        shared[p + "Qp"] = f32(inp[p + "WQ"][:, :, HEAD_PERM])
        shared[p + "Kp"] = f32(inp[p + "WK"][:, :, HEAD_PERM])
        shared[p + "Vp"] = f32(inp[p + "WV"][:, :, HEAD_PERM])
        shared[p + "O"] = f32(inp[p + "WO"])
        shared[p + "Fi"] = f32(inp[p + "Fi"])
        shared[p + "Fo"] = f32(inp[p + "Fo"])
        shared[p + "Qbp"] = f32(inp[p + "WQb"][:, HEAD_PERM]).reshape(L, EC, P)
        shared[p + "Kbp"] = f32(inp[p + "WKb"][:, HEAD_PERM]).reshape(L, EC, P)
        shared[p + "Vbr"] = f32(inp[p + "WVb"][:, HEAD_PERM]).reshape(L, 1, ED)
        shared[p + "Ob"] = f32(inp[p + "WOb"]).reshape(L, EC, P)
        shared[p + "Fib"] = f32(inp[p + "Fib"]).reshape(L, FC, P)
        shared[p + "Fob"] = f32(inp[p + "Fob"]).reshape(L, EC, P)
        shared[p + "g"] = f32(np.concatenate(
            [inp[p + "na"].reshape(L, EC, P),
             inp[p + "nb"].reshape(L, EC, P)], axis=1))

    causal_kq = np.tril(np.ones((TS, TS), np.float32)).T  # [k, q]: keep k<=q

    in_maps = []
    for c in range(NCORES):
        b = c % BS
        m = dict(shared)
        m["semb_g"] = f32(inp["src_emb"][src[b]])
        m["temb_g"] = f32(inp["tgt_emb"][tgt[b]])
        m["pe_cols"] = f32(pe[b].reshape(EC, P))
        m["encm"] = f32(np.where(src[b] != PAD, 0.0, NEG).reshape(SC, P))
        keep = (tgt[b] != PAD).astype(np.float32)[:, None] * causal_kq
        m["decm8"] = f32(np.where(keep > 0, 0.0, -1e9).reshape(TC, P, TS))
        m["gWs"] = f32(inp["gW"][:, c * VSL:(c + 1) * VSL])
        m["gbs"] = f32(inp["gb"][c * VSL:(c + 1) * VSL].reshape(1, VSL))
        in_maps.append(m)

    res = run_bass_kernel_spmd(nc, in_maps, list(range(NCORES)))
    LAST_RESULTS = res
    out = np.concatenate(
        [res.results[c]["out"].reshape(BS, TS, VSL) for c in range(NCORES)],
        axis=-1)
    return out.astype(np.float32)
